# revision 17
# baseline (speedup 1.0000x reference)
"""Trainium2 Bass kernel: nearest triangle (closest point on mesh) brute force.

kernel(triangles [1,1024,3,3] f32, points [1,16384,3] f32) ->
    (distances [1,16384] f32, closest_points [1,16384,3] f32,
     closest_faces [1,16384] int32)

Sharding: data-parallel over points across 8 NeuronCores (2048 points/core);
triangles (and derived per-triangle constants) replicated on every core.
"""

import numpy as np

N_CORES = 8
F = 1024
P_TOTAL = 16384
P_LOC = P_TOTAL // N_CORES          # 2048
PTILES = P_LOC // 128               # 16 point tiles of 128 per core
FH = F // 2                         # 512, PSUM-bank-sized chunk

_PROGRAM_CACHE = {}


def _build_program(ptiles=PTILES):
    """Build + compile the (input-independent) Bass program once."""
    import concourse.bacc as bacc
    import concourse.mybir as mybir
    from concourse import tile

    dt = mybir.dt
    alu = mybir.AluOpType
    AF = mybir.ActivationFunctionType

    nc = bacc.Bacc("TRN2", target_bir_lowering=False, debug=False,
                   num_devices=N_CORES)

    d_crows = nc.dram_tensor("crows", [16, F], dt.float32, kind="ExternalInput")
    d_m6 = nc.dram_tensor("m6", [24, F], dt.float32, kind="ExternalInput")
    d_ptsT = nc.dram_tensor("ptsT", [4, P_LOC], dt.float32, kind="ExternalInput")
    d_pcol = nc.dram_tensor("pcol", [128, 3 * PTILES], dt.float32,
                            kind="ExternalInput")
    d_od = nc.dram_tensor("od", [128, PTILES], dt.float32, kind="ExternalOutput")
    d_ocp = nc.dram_tensor("ocp", [128, 3 * PTILES], dt.float32,
                           kind="ExternalOutput")
    d_of = nc.dram_tensor("of", [128, PTILES], dt.float32, kind="ExternalOutput")

    f32 = dt.float32

    with tile.TileContext(nc) as tc:
        with (
            tc.tile_pool(name="const", bufs=1) as cpool,
            tc.tile_pool(name="dwork", bufs=1) as dpool,
            tc.tile_pool(name="work", bufs=1) as wpool,
            tc.tile_pool(name="small", bufs=2) as spool,
            tc.tile_pool(name="psum", bufs=1, space="PSUM") as ppool,
        ):
            V = nc.vector
            G = nc.gpsimd
            S = nc.scalar

            def WT(tag, bufs=None, dtype=None):
                return wpool.tile([128, F], dtype or f32, tag=tag, name=tag,
                                  bufs=bufs)

            # ---- stage inputs ----
            # PE requires operand base partition in {0,32,64}: every matmul
            # operand tile is base-0.
            m6t = []
            for j in range(6):
                mj = cpool.tile([4, F], f32, tag=f"m6_{j}", name=f"m6_{j}")
                G.dma_start(mj[:], d_m6[4 * j:4 * j + 4, :])
                m6t.append(mj)
            pcol = cpool.tile([128, 3 * PTILES], f32, tag="pcol", name="pcol")
            G.dma_start(pcol[:], d_pcol[:])

            ones = cpool.tile([1, 128], f32, tag="ones", name="ones")
            G.memset(ones[:], 1.0)

            # ---- broadcast per-triangle constant rows to [128, F] tiles ----
            # crows rows: ABx ABy ABz ACx ACy ACz Ax Ay Az RAB RAC RBC RDEN IOTA
            CN = ["ABx", "ABy", "ABz", "ACx", "ACy", "ACz",
                  "Ax", "Ay", "Az", "RAB", "RAC", "RBC", "RDEN", "IOTA"]
            CB = {}
            for i, nm in enumerate(CN):
                cs = wpool.tile([1, F], f32, tag="pA", name="crowstage")
                G.dma_start(cs[:], d_crows[i:i + 1, :])
                ct = cpool.tile([128, F], f32, tag=nm, name=nm)
                for h in range(2):
                    ps = ppool.tile([128, FH], f32, tag="bps", name="bps", bufs=2)
                    nc.tensor.matmul(ps[:], ones[:],
                                     cs[0:1, h * FH:(h + 1) * FH])
                    S.copy(ct[:, h * FH:(h + 1) * FH], ps[:])
                CB[nm] = ct
            zero = cpool.tile([128, F], f32, tag="ZERO", name="ZERO")
            G.memset(zero[:], 0.0)
            onet = cpool.tile([128, F], f32, tag="ONE", name="ONE")
            G.memset(onet[:], 1.0)

            od = cpool.tile([128, PTILES], f32, tag="od", name="od")
            ocp = cpool.tile([128, 3 * PTILES], f32, tag="ocp", name="ocp")
            of = cpool.tile([128, PTILES], f32, tag="of", name="of")

            ABb = [CB["ABx"], CB["ABy"], CB["ABz"]]
            ACb = [CB["ACx"], CB["ACy"], CB["ACz"]]
            Ab = [CB["Ax"], CB["Ay"], CB["Az"]]

            for t in range(ptiles):
                # ---- d1..d6 via PE homogeneous matmuls ----
                pT = cpool.tile([4, 128], f32, tag="pT", name="pT", bufs=2)
                G.dma_start(pT[:], d_ptsT[0:4, t * 128:(t + 1) * 128])
                ds = []
                for j in range(6):
                    dj = dpool.tile([128, F], f32, tag=f"d{j}", name=f"d{j}")
                    for h in range(2):
                        ps = ppool.tile([128, FH], f32, tag="dps", name="dps", bufs=6)
                        nc.tensor.matmul(
                            ps[:], pT[:],
                            m6t[j][0:4, h * FH:(h + 1) * FH])
                        S.copy(dj[:, h * FH:(h + 1) * FH], ps[:])
                    ds.append(dj)
                d1, d2, d3, d4, d5, d6 = ds

                # ---- va/vb/vc (all out-of-place) ----
                pA = WT("pA"); V.tensor_tensor(pA[:], d1[:], d4[:], alu.mult)
                pB = WT("pB"); V.tensor_tensor(pB[:], d3[:], d2[:], alu.mult)
                vc = WT("vc"); V.tensor_tensor(vc[:], pA[:], pB[:], alu.subtract)
                pC = WT("pA"); V.tensor_tensor(pC[:], d5[:], d2[:], alu.mult)
                pD = WT("pB"); V.tensor_tensor(pD[:], d1[:], d6[:], alu.mult)
                vb = WT("vb"); V.tensor_tensor(vb[:], pC[:], pD[:], alu.subtract)
                pE = WT("pA"); V.tensor_tensor(pE[:], d3[:], d6[:], alu.mult)
                pF = WT("pB"); V.tensor_tensor(pF[:], d5[:], d4[:], alu.mult)
                va = WT("va"); V.tensor_tensor(va[:], pE[:], pF[:], alu.subtract)

                # ---- interior v, w with NR reciprocal of fp denom ----
                s1 = WT("pA"); V.tensor_tensor(s1[:], va[:], vb[:], alu.add)
                dn = WT("pB"); V.tensor_tensor(dn[:], s1[:], vc[:], alu.add)
                u0 = WT("pA"); V.tensor_tensor(u0[:], dn[:], CB["RDEN"][:], alu.mult)
                u1 = WT("u1"); S.activation(u1[:], u0[:], AF.Copy, bias=2.0, scale=-1.0)
                r1 = WT("pB"); V.tensor_tensor(r1[:], CB["RDEN"][:], u1[:], alu.mult)
                v = WT("v"); V.tensor_tensor(v[:], vb[:], r1[:], alu.mult)
                w = WT("w"); V.tensor_tensor(w[:], vc[:], r1[:], alu.mult)

                # ---- edge quantities ----
                tnum = WT("tnum"); V.tensor_tensor(tnum[:], d4[:], d3[:], alu.subtract)
                tden = WT("tden"); V.tensor_tensor(tden[:], d5[:], d6[:], alu.subtract)
                tt_ = WT("tt"); V.tensor_tensor(tt_[:], tnum[:], CB["RBC"][:], alu.mult)
                omt = WT("omt"); S.activation(omt[:], tt_[:], AF.Copy, bias=1.0, scale=-1.0)
                we = WT("we"); V.tensor_tensor(we[:], d2[:], CB["RAC"][:], alu.mult)
                ve = WT("ve"); V.tensor_tensor(ve[:], d1[:], CB["RAB"][:], alu.mult)

                # ---- masks (negations + relu-cast on ACT, mins on V) ----
                def relu_mask(nm, mn, scale=1e30):
                    mk = wpool.tile([128, F], dt.uint8, tag=nm, name=nm)
                    S.activation(mk[:], mn[:], AF.Relu, scale=scale)
                    return mk

                nva = WT("ngA"); S.mul(nva[:], va[:], -1.0)
                mn1 = WT("mnA"); V.tensor_tensor(mn1[:], nva[:], tnum[:], alu.min)
                mn2 = WT("mnB"); V.tensor_tensor(mn2[:], mn1[:], tden[:], alu.min)
                m_bc = relu_mask("m_bc", mn2)

                nvb = WT("ngB"); S.mul(nvb[:], vb[:], -1.0)
                nd6 = WT("ngA"); S.mul(nd6[:], d6[:], -1.0)
                mn3 = WT("mnA"); V.tensor_tensor(mn3[:], nvb[:], d2[:], alu.min)
                mn4 = WT("mnB"); V.tensor_tensor(mn4[:], mn3[:], nd6[:], alu.min)
                m_ac = relu_mask("m_ac", mn4)

                nvc = WT("ngB"); S.mul(nvc[:], vc[:], -1.0)
                nd3 = WT("ngA"); S.mul(nd3[:], d3[:], -1.0)
                mn5 = WT("mnA"); V.tensor_tensor(mn5[:], nvc[:], d1[:], alu.min)
                mn6 = WT("mnB"); V.tensor_tensor(mn6[:], mn5[:], nd3[:], alu.min)
                m_ab = relu_mask("m_ab", mn6)

                ntd = WT("ngB"); S.mul(ntd[:], tden[:], -1.0)
                mn7 = WT("mnA"); V.tensor_tensor(mn7[:], ntd[:], d6[:], alu.min)
                m_c = relu_mask("m_c", mn7)

                ntn = WT("ngA"); S.mul(ntn[:], tnum[:], -1.0)
                mn8 = WT("mnB"); V.tensor_tensor(mn8[:], ntn[:], d3[:], alu.min)
                m_b = relu_mask("m_b", mn8)

                mn9 = WT("mnA"); V.tensor_tensor(mn9[:], d1[:], d2[:], alu.max)
                m_a = relu_mask("m_a", mn9, scale=-1e30)

                # ---- select cascade (priority order) ----
                V.copy_predicated(v[:], m_bc[:], omt[:])
                V.copy_predicated(w[:], m_bc[:], tt_[:])
                V.copy_predicated(v[:], m_ac[:], zero[:])
                V.copy_predicated(w[:], m_ac[:], we[:])
                V.copy_predicated(v[:], m_ab[:], ve[:])
                V.copy_predicated(w[:], m_ab[:], zero[:])
                V.copy_predicated(v[:], m_c[:], zero[:])
                V.copy_predicated(w[:], m_c[:], onet[:])
                V.copy_predicated(v[:], m_b[:], onet[:])
                V.copy_predicated(w[:], m_b[:], zero[:])
                V.copy_predicated(v[:], m_a[:], zero[:])
                V.copy_predicated(w[:], m_a[:], zero[:])

                # ---- closest point + squared residual (out-of-place) ----
                cks = []
                sqs = []
                sq_tags = ["ngA", "ngB", "u1"]
                for k in range(3):
                    c1 = WT("mnA"); V.tensor_tensor(c1[:], v[:], ABb[k][:], alu.mult)
                    c2 = WT("mnB"); V.tensor_tensor(c2[:], c1[:], Ab[k][:], alu.add)
                    c3 = WT("mnA"); V.tensor_tensor(c3[:], w[:], ACb[k][:], alu.mult)
                    ck = WT(f"ck{k}")
                    V.tensor_tensor(ck[:], c2[:], c3[:], alu.add)
                    sq = wpool.tile([128, F], f32, tag=sq_tags[k], name=f"sq{k}")
                    S.activation(sq[:], ck[:], AF.Square,
                                 bias=pcol[:, 3 * t + k:3 * t + k + 1],
                                 scale=-1.0)
                    cks.append(ck)
                    sqs.append(sq)

                n1 = WT("mnA"); V.tensor_tensor(n1[:], sqs[0][:], sqs[1][:], alu.add)
                nd0 = WT("mnB"); V.tensor_tensor(nd0[:], n1[:], sqs[2][:], alu.add)
                nd = WT("tnum"); S.mul(nd[:], nd0[:], -1.0)

                # ---- argmin via top-8 max of -dist2 ----
                max8 = spool.tile([128, 8], f32, tag="max8", name="max8")
                V.max(max8[:], nd[:])
                idx8 = spool.tile([128, 8], dt.uint32, tag="idx8", name="idx8")
                V.max_index(idx8[:], max8[:], nd[:])
                fx = spool.tile([128, 1], f32, tag="fx", name="fx")
                V.tensor_copy(fx[:], idx8[:, 0:1])
                oh = WT("tt")
                V.tensor_scalar(oh[:], CB["IOTA"][:], fx[:], None, alu.is_equal)

                scr = WT("omt")
                for k in range(3):
                    msk = WT("mnA" if k != 1 else "mnB")
                    V.tensor_tensor(msk[:], oh[:], cks[k][:], alu.mult)
                    S.activation(scr[:], msk[:], AF.Copy,
                                 accum_out=ocp[:, 3 * t + k:3 * t + k + 1])

                S.mul(od[:, t:t + 1], max8[:, 0:1], -1.0)
                S.copy(of[:, t:t + 1], fx[:])

            G.dma_start(d_od[:], od[:])
            G.dma_start(d_ocp[:], ocp[:])
            G.dma_start(d_of[:], of[:])

    nc.compile()
    return nc


def _host_prep(triangles, points):
    """Per-triangle constants + per-core point shards (numpy fp32)."""
    f32 = np.float32
    tri = np.ascontiguousarray(triangles[0], dtype=f32)   # [F,3,3]
    pts = np.ascontiguousarray(points[0], dtype=f32)      # [P,3]

    A = tri[:, 0, :]; B = tri[:, 1, :]; C = tri[:, 2, :]
    AB = B - A
    AC = C - A

    def dot3(x, y):
        t = x * y
        return (t[:, 0] + t[:, 1]) + t[:, 2]

    ABdA = dot3(AB, A); ACdA = dot3(AC, A)
    ABdB = dot3(AB, B); ACdB = dot3(AC, B)
    ABdC = dot3(AB, C); ACdC = dot3(AC, C)

    AB64 = AB.astype(np.float64); AC64 = AC.astype(np.float64)
    BC64 = (C - B).astype(np.float64)
    cr = np.cross(AB64, AC64)
    R_AB = (1.0 / (AB64 ** 2).sum(1)).astype(f32)
    R_AC = (1.0 / (AC64 ** 2).sum(1)).astype(f32)
    R_BC = (1.0 / (BC64 ** 2).sum(1)).astype(f32)
    R_DEN = (1.0 / (cr ** 2).sum(1)).astype(f32)

    crows = np.zeros((16, F), f32)
    for i, row in enumerate([AB[:, 0], AB[:, 1], AB[:, 2],
                             AC[:, 0], AC[:, 1], AC[:, 2],
                             A[:, 0], A[:, 1], A[:, 2],
                             R_AB, R_AC, R_BC, R_DEN,
                             np.arange(F, dtype=f32)]):
        crows[i] = row

    m6 = np.zeros((24, F), f32)
    mats = [(AB, ABdA), (AC, ACdA), (AB, ABdB), (AC, ACdB), (AB, ABdC), (AC, ACdC)]
    for j, (E, c) in enumerate(mats):
        m6[4 * j + 0] = E[:, 0]
        m6[4 * j + 1] = E[:, 1]
        m6[4 * j + 2] = E[:, 2]
        m6[4 * j + 3] = -c

    in_maps = []
    for cidx in range(N_CORES):
        pc = pts[cidx * P_LOC:(cidx + 1) * P_LOC]          # [2048,3]
        ptsT = np.empty((4, P_LOC), f32)
        ptsT[0] = pc[:, 0]; ptsT[1] = pc[:, 1]; ptsT[2] = pc[:, 2]; ptsT[3] = 1.0
        pcol = np.empty((128, 3 * PTILES), f32)
        for t in range(PTILES):
            blk = pc[t * 128:(t + 1) * 128]                # [128,3]
            pcol[:, 3 * t:3 * t + 3] = blk
        in_maps.append({
            "crows": crows, "m6": m6, "ptsT": ptsT, "pcol": pcol,
        })
    return in_maps


def kernel(triangles, points):
    from concourse.bass_utils import run_bass_kernel_spmd

    if "nc" not in _PROGRAM_CACHE:
        _PROGRAM_CACHE["nc"] = _build_program()
    nc = _PROGRAM_CACHE["nc"]

    in_maps = _host_prep(triangles, points)
    res = run_bass_kernel_spmd(nc, in_maps, list(range(N_CORES)))

    distances = np.empty((1, P_TOTAL), np.float32)
    closest_points = np.empty((1, P_TOTAL, 3), np.float32)
    closest_faces = np.empty((1, P_TOTAL), np.int32)
    for cidx in range(N_CORES):
        r = res.results[cidx]
        od, ocp, of = r["od"], r["ocp"], r["of"]
        base = cidx * P_LOC
        for t in range(PTILES):
            sl = slice(base + t * 128, base + (t + 1) * 128)
            distances[0, sl] = od[:, t]
            closest_points[0, sl, :] = ocp[:, 3 * t:3 * t + 3]
            closest_faces[0, sl] = of[:, t].astype(np.int32)
    return distances, closest_points, closest_faces
